# revision 25
# baseline (speedup 1.0000x reference)
"""Additive attention (Bahdanau) on 8 TRN2 NeuronCores, data-parallel over batch.

Reference computation (per batch row b):
    w1q   = W1 @ query[b]                      # [AD]
    w2k   = W2 @ keys[b].T                     # [AD, S]
    comb  = tanh(w1q[:, None] + w2k)           # [AD, S]
    score = v @ comb                           # [S]
    out   = softmax(where(mask, score, -inf))  # [S]

Shapes: B=32, S=2048, D=AD=512. Each of the 8 cores handles 4 batch rows;
weights are replicated, no collectives are needed.

Device kernel layout choices:
  - keys are fed pre-transposed per batch ([D, S]) so the contraction dim d
    sits on SBUF partitions for the TensorEngine.
  - matmuls run in bf16 (f32 PSUM accumulate): full PE rate, half the DMA
    bytes, and the LDWEIGHTS+MATMUL pair gives two semaphore-wait slots
    (self-loading 4-byte matmuls only get one, which the Tile-emitted waits
    overflow).
  - main matmul produces w2k in [a, s] layout; tanh + per-partition w1q bias
    is one ScalarE activation reading PSUM.
  - the v-dot is a second matmul whose stationary operand vsel[:, b] has
    column j equal to the v-chunk iff j == b, so all four batches accumulate
    into one [4, 512] PSUM tile with row j = batch j (engine ops cannot start
    at partition offsets that are not multiples of 32, so per-row copies are
    not an option).
  - scores are bounded (|score| <= ||v||_1), so softmax skips the max
    subtraction: weights = mask * e^s / sum(mask * e^s), with exp running
    incrementally per s-tile and the row-sum fused into the masking multiply
    (accum_out).
"""

import numpy as np

B, S, D, AD = 32, 2048, 512, 512
NCORES = 8
BPC = B // NCORES  # batch rows per core
P = 128
KC = D // P   # contraction chunks
MC = AD // P  # a-chunks
ST = 4        # s-tiles per row
SW = S // ST  # 512
VW = 512      # v-matmul / psum-bank width

_CACHE = {}


def _build_nc():
    import concourse.mybir as mybir
    from concourse import bacc
    from concourse.tile import TileContext

    f32 = mybir.dt.float32
    bf16 = mybir.dt.bfloat16
    AF = mybir.ActivationFunctionType
    MUL = mybir.AluOpType.mult

    nc = bacc.Bacc()
    kT = nc.declare_dram_parameter("kT", [BPC, D, S], bf16, isOutput=False)
    w2t = nc.declare_dram_parameter("w2t", [D, AD], bf16, isOutput=False)
    w1t = nc.declare_dram_parameter("w1t", [D, AD], bf16, isOutput=False)
    qT = nc.declare_dram_parameter("qT", [D, BPC], bf16, isOutput=False)
    vsel = nc.declare_dram_parameter("vsel", [P, BPC, MC, BPC], bf16, isOutput=False)
    m01 = nc.declare_dram_parameter("m01", [BPC, S], f32, isOutput=False)
    out = nc.declare_dram_parameter("out", [BPC, S], f32, isOutput=True)

    with TileContext(nc) as tc:
        with (
            tc.tile_pool(name="singles", bufs=1) as singles,
            tc.tile_pool(name="ktp", bufs=4) as ktp,
            tc.tile_pool(name="combp", bufs=4) as combp,
            tc.tile_pool(name="psmain", bufs=4, space="PSUM") as psmain,
            tc.tile_pool(name="psaux", bufs=3, space="PSUM") as psaux,
        ):
            # HAM warm-up: matmuls on garbage SBUF keep the PE busy and
            # un-throttled while the first keys tiles stream in; the PSUM
            # results are never read.
            wu_a = singles.tile([P, P], bf16)
            wu_b = singles.tile([P, VW], bf16)
            nc.vector.memset(wu_a, 0.0)
            nc.vector.memset(wu_b, 0.0)
            for _ in range(2):
                wu_ps = psmain.tile([P, SW], f32, tag="pc")
                for _ in range(8):
                    nc.tensor.matmul(wu_ps[:, :VW], lhsT=wu_a, rhs=wu_b, start=True, stop=True)

            # first keys tile + weights, in consumption order
            kt_first = ktp.tile([P, KC, SW], bf16, tag="kt")
            nc.sync.dma_start(
                out=kt_first,
                in_=kT[0].rearrange("(kc p) s -> p kc s", p=P)[:, :, 0:SW],
            )
            w2t_sb = singles.tile([P, KC, AD], bf16)
            nc.sync.dma_start(out=w2t_sb, in_=w2t.ap().rearrange("(kc p) a -> p kc a", p=P))
            w1t_sb = singles.tile([P, KC, AD], bf16)
            nc.sync.dma_start(out=w1t_sb, in_=w1t.ap().rearrange("(kc p) a -> p kc a", p=P))
            qT_sb = singles.tile([P, KC, BPC], bf16)
            nc.sync.dma_start(out=qT_sb, in_=qT.ap().rearrange("(kc p) b -> p kc b", p=P))
            vsel_sb = singles.tile([P, BPC, MC, BPC], bf16)
            nc.sync.dma_start(out=vsel_sb, in_=vsel.ap())
            m01_sb = singles.tile([BPC, S], f32)
            nc.sync.dma_start(out=m01_sb, in_=m01.ap())

            # w1q[a, b] = sum_d W1[a, d] * query[b, d], laid out [p, m, b]
            w1q_sb = singles.tile([P, MC, BPC], f32)
            for m in range(MC):
                pw = psaux.tile([P, BPC], f32, tag="aux")
                for k in range(KC):
                    nc.tensor.matmul(
                        pw,
                        lhsT=w1t_sb[:, k, m * P : (m + 1) * P],
                        rhs=qT_sb[:, k, :],
                        start=(k == 0),
                        stop=(k == KC - 1),
                    )
                nc.vector.tensor_copy(w1q_sb[:, m, :], pw)

            prob = singles.tile([BPC, S], f32)
            sums = singles.tile([BPC, ST], f32)

            for st in range(ST):
                sc_ps = psaux.tile([BPC, SW], f32, tag="aux")
                pending_v = []
                for b in range(BPC):
                    if st == 0 and b == 0:
                        kt_t = kt_first
                    else:
                        kt_t = ktp.tile([P, KC, SW], bf16, tag="kt")
                        nc.sync.dma_start(
                            out=kt_t,
                            in_=kT[b].rearrange("(kc p) s -> p kc s", p=P)[
                                :, :, st * SW : (st + 1) * SW
                            ],
                        )
                    for m in range(MC):
                        pc = psmain.tile([P, SW], f32, tag="pc")
                        for k in range(KC):
                            nc.tensor.matmul(
                                pc,
                                lhsT=w2t_sb[:, k, m * P : (m + 1) * P],
                                rhs=kt_t[:, k, :],
                                start=(k == 0),
                                stop=(k == KC - 1),
                            )
                        for pv in pending_v:
                            nc.tensor.matmul(*pv[0], **pv[1])
                        pending_v = []
                        comb = combp.tile([P, SW], bf16, tag="comb")
                        nc.scalar.activation(
                            comb, pc, AF.Tanh, bias=w1q_sb[:, m, b : b + 1]
                        )
                        pending_v.append(
                            (
                                (sc_ps,),
                                dict(
                                    lhsT=vsel_sb[:, b, m, :],
                                    rhs=comb,
                                    start=(b == 0 and m == 0),
                                    stop=(b == BPC - 1 and m == MC - 1),
                                ),
                            )
                        )
                # flush the final b's v-run before exp reads sc_ps
                for pv in pending_v:
                    nc.tensor.matmul(*pv[0], **pv[1])
                pending_v = []
                # scores are bounded (|score| <= ||v||_1 ~ 18) so exp needs no
                # max subtraction; masked softmax = mask * e^s / sum(mask * e^s)
                e_st = combp.tile([BPC, SW], f32, tag="est")
                nc.scalar.activation(e_st, sc_ps, AF.Exp)
                nc.vector.scalar_tensor_tensor(
                    prob[:, st * SW : (st + 1) * SW],
                    e_st,
                    1.0,
                    m01_sb[:, st * SW : (st + 1) * SW],
                    MUL,
                    MUL,
                    accum_out=sums[:, st : st + 1],
                )

            tot = singles.tile([BPC, 1], f32)
            nc.vector.reduce_sum(tot, sums, axis=mybir.AxisListType.X)
            rs = singles.tile([BPC, 1], f32)
            nc.vector.reciprocal(rs, tot)
            outw = singles.tile([BPC, S], f32)
            nc.vector.tensor_scalar_mul(outw, prob, rs)
            nc.sync.dma_start(out=out.ap(), in_=outw)

    nc.finalize()
    return nc


def get_nc():
    if "nc" not in _CACHE:
        _CACHE["nc"] = _build_nc()
    return _CACHE["nc"]


def prep_in_maps(query, keys, mask, W1, W2, v):
    query = np.ascontiguousarray(np.asarray(query, dtype=np.float32))
    keys = np.asarray(keys, dtype=np.float32)
    mask = np.asarray(mask)
    W1 = np.asarray(W1, dtype=np.float32)
    W2 = np.asarray(W2, dtype=np.float32)
    v = np.asarray(v, dtype=np.float32)

    import ml_dtypes

    bf = ml_dtypes.bfloat16
    w1t = np.ascontiguousarray(W1.T).astype(bf)
    w2t = np.ascontiguousarray(W2.T).astype(bf)
    # vsel[p, b, m, j] = v[m*128 + p] if j == b else 0
    vsel = np.zeros((P, BPC, MC, BPC), dtype=np.float32)
    vchunk = v.reshape(MC, P).T  # [p, m]
    for b in range(BPC):
        vsel[:, b, :, b] = vchunk
    vsel = vsel.astype(bf)
    m01f = mask.astype(np.float32)

    in_maps = []
    for c in range(NCORES):
        sl = slice(c * BPC, (c + 1) * BPC)
        in_maps.append(
            {
                "kT": np.ascontiguousarray(keys[sl].transpose(0, 2, 1)).astype(bf),
                "w2t": w2t,
                "w1t": w1t,
                "qT": np.ascontiguousarray(query[sl].T).astype(bf),
                "vsel": vsel,
                "m01": np.ascontiguousarray(m01f[sl]),
            }
        )
    return in_maps


def run(query, keys, mask, W1, W2, v, trace=False):
    """Run on the 8 NeuronCores; returns (output, BassKernelResults)."""
    from concourse.bass_utils import run_bass_kernel_spmd

    nc = get_nc()
    in_maps = prep_in_maps(query, keys, mask, W1, W2, v)
    res = run_bass_kernel_spmd(nc, in_maps, core_ids=list(range(NCORES)), trace=trace)
    outs = [np.asarray(res.results[c]["out"]) for c in range(NCORES)]
    full = np.concatenate(outs, axis=0).astype(np.float32)
    return full, res


def kernel(query, keys, mask, W1, W2, v):
    full, _ = run(query, keys, mask, W1, W2, v, trace=False)
    return full


# revision 26
# speedup vs baseline: 1.0019x; 1.0019x over previous
"""Additive attention (Bahdanau) on 8 TRN2 NeuronCores, data-parallel over batch.

Reference computation (per batch row b):
    w1q   = W1 @ query[b]                      # [AD]
    w2k   = W2 @ keys[b].T                     # [AD, S]
    comb  = tanh(w1q[:, None] + w2k)           # [AD, S]
    score = v @ comb                           # [S]
    out   = softmax(where(mask, score, -inf))  # [S]

Shapes: B=32, S=2048, D=AD=512. Each of the 8 cores handles 4 batch rows;
weights are replicated, no collectives are needed.

Device kernel layout choices:
  - keys are fed pre-transposed per batch ([D, S]) so the contraction dim d
    sits on SBUF partitions for the TensorEngine.
  - matmuls run in bf16 (f32 PSUM accumulate): full PE rate, half the DMA
    bytes, and the LDWEIGHTS+MATMUL pair gives two semaphore-wait slots
    (self-loading 4-byte matmuls only get one, which the Tile-emitted waits
    overflow).
  - main matmul produces w2k in [a, s] layout; tanh + per-partition w1q bias
    is one ScalarE activation reading PSUM.
  - the v-dot is a second matmul whose stationary operand vsel[:, b] has
    column j equal to the v-chunk iff j == b, so all four batches accumulate
    into one [4, 512] PSUM tile with row j = batch j (engine ops cannot start
    at partition offsets that are not multiples of 32, so per-row copies are
    not an option).
  - scores are bounded (|score| <= ||v||_1), so softmax skips the max
    subtraction: weights = mask * e^s / sum(mask * e^s), with exp running
    incrementally per s-tile and the row-sum fused into the masking multiply
    (accum_out).
"""

import numpy as np

B, S, D, AD = 32, 2048, 512, 512
NCORES = 8
BPC = B // NCORES  # batch rows per core
P = 128
KC = D // P   # contraction chunks
MC = AD // P  # a-chunks
ST = 4        # s-tiles per row
SW = S // ST  # 512
VW = 512      # v-matmul / psum-bank width

_CACHE = {}


def _build_nc():
    import concourse.mybir as mybir
    from concourse import bacc
    from concourse.tile import TileContext

    f32 = mybir.dt.float32
    bf16 = mybir.dt.bfloat16
    AF = mybir.ActivationFunctionType
    MUL = mybir.AluOpType.mult

    nc = bacc.Bacc()
    kT = nc.declare_dram_parameter("kT", [BPC, D, S], bf16, isOutput=False)
    w2t = nc.declare_dram_parameter("w2t", [D, AD], bf16, isOutput=False)
    w1t = nc.declare_dram_parameter("w1t", [D, AD], bf16, isOutput=False)
    qT = nc.declare_dram_parameter("qT", [D, BPC], bf16, isOutput=False)
    vsel = nc.declare_dram_parameter("vsel", [P, BPC, MC, BPC], bf16, isOutput=False)
    m01 = nc.declare_dram_parameter("m01", [BPC, S], f32, isOutput=False)
    out = nc.declare_dram_parameter("out", [BPC, S], f32, isOutput=True)

    with TileContext(nc) as tc:
        with (
            tc.tile_pool(name="singles", bufs=1) as singles,
            tc.tile_pool(name="ktp", bufs=4) as ktp,
            tc.tile_pool(name="combp", bufs=4) as combp,
            tc.tile_pool(name="psmain", bufs=4, space="PSUM") as psmain,
            tc.tile_pool(name="psaux", bufs=3, space="PSUM") as psaux,
        ):
            # HAM warm-up: matmuls on garbage SBUF keep the PE busy and
            # un-throttled while the first keys tiles stream in; the PSUM
            # results are never read.
            wu_a = singles.tile([P, P], bf16)
            wu_b = singles.tile([P, VW], bf16)
            nc.vector.memset(wu_a, 0.0)
            nc.vector.memset(wu_b, 0.0)
            for _ in range(2):
                wu_ps = psmain.tile([P, SW], f32, tag="pc")
                for _ in range(8):
                    nc.tensor.matmul(wu_ps[:, :VW], lhsT=wu_a, rhs=wu_b, start=True, stop=True)

            # first keys tile + weights, in consumption order
            kt_first = ktp.tile([P, KC, SW], bf16, tag="kt")
            nc.sync.dma_start(
                out=kt_first,
                in_=kT[0].rearrange("(kc p) s -> p kc s", p=P)[:, :, 0:SW],
            )
            w2t_sb = singles.tile([P, KC, AD], bf16)
            nc.sync.dma_start(out=w2t_sb, in_=w2t.ap().rearrange("(kc p) a -> p kc a", p=P))
            w1t_sb = singles.tile([P, KC, AD], bf16)
            nc.sync.dma_start(out=w1t_sb, in_=w1t.ap().rearrange("(kc p) a -> p kc a", p=P))
            qT_sb = singles.tile([P, KC, BPC], bf16)
            nc.sync.dma_start(out=qT_sb, in_=qT.ap().rearrange("(kc p) b -> p kc b", p=P))
            vsel_sb = singles.tile([P, BPC, MC, BPC], bf16)
            nc.sync.dma_start(out=vsel_sb, in_=vsel.ap())
            m01_sb = singles.tile([BPC, S], f32)
            nc.sync.dma_start(out=m01_sb, in_=m01.ap())

            w1q_sb = singles.tile([P, MC, BPC], f32)

            def emit_w1q():
                # w1q[a, b] = sum_d W1[a, d] * query[b, d], laid out [p, m, b].
                # Emitted after the first main matmul group: the PE queue is
                # strict FIFO, so putting these (which wait on the later
                # w1t/qT DMAs) first would stall the main matmuls behind them.
                for m in range(MC):
                    pw = psaux.tile([P, BPC], f32, tag="aux")
                    for k in range(KC):
                        nc.tensor.matmul(
                            pw,
                            lhsT=w1t_sb[:, k, m * P : (m + 1) * P],
                            rhs=qT_sb[:, k, :],
                            start=(k == 0),
                            stop=(k == KC - 1),
                        )
                    nc.vector.tensor_copy(w1q_sb[:, m, :], pw)

            prob = singles.tile([BPC, S], f32)
            sums = singles.tile([BPC, ST], f32)

            for st in range(ST):
                sc_ps = psaux.tile([BPC, SW], f32, tag="aux")
                pending_v = []
                for b in range(BPC):
                    if st == 0 and b == 0:
                        kt_t = kt_first
                    else:
                        kt_t = ktp.tile([P, KC, SW], bf16, tag="kt")
                        nc.sync.dma_start(
                            out=kt_t,
                            in_=kT[b].rearrange("(kc p) s -> p kc s", p=P)[
                                :, :, st * SW : (st + 1) * SW
                            ],
                        )
                    for m in range(MC):
                        pc = psmain.tile([P, SW], f32, tag="pc")
                        for k in range(KC):
                            nc.tensor.matmul(
                                pc,
                                lhsT=w2t_sb[:, k, m * P : (m + 1) * P],
                                rhs=kt_t[:, k, :],
                                start=(k == 0),
                                stop=(k == KC - 1),
                            )
                        if st == 0 and b == 0 and m == 0:
                            emit_w1q()
                        for pv in pending_v:
                            nc.tensor.matmul(*pv[0], **pv[1])
                        pending_v = []
                        comb = combp.tile([P, SW], bf16, tag="comb")
                        nc.scalar.activation(
                            comb, pc, AF.Tanh, bias=w1q_sb[:, m, b : b + 1]
                        )
                        pending_v.append(
                            (
                                (sc_ps,),
                                dict(
                                    lhsT=vsel_sb[:, b, m, :],
                                    rhs=comb,
                                    start=(b == 0 and m == 0),
                                    stop=(b == BPC - 1 and m == MC - 1),
                                ),
                            )
                        )
                # flush the final b's v-run before exp reads sc_ps
                for pv in pending_v:
                    nc.tensor.matmul(*pv[0], **pv[1])
                pending_v = []
                # scores are bounded (|score| <= ||v||_1 ~ 18) so exp needs no
                # max subtraction; masked softmax = mask * e^s / sum(mask * e^s)
                e_st = combp.tile([BPC, SW], f32, tag="est")
                nc.scalar.activation(e_st, sc_ps, AF.Exp)
                nc.vector.scalar_tensor_tensor(
                    prob[:, st * SW : (st + 1) * SW],
                    e_st,
                    1.0,
                    m01_sb[:, st * SW : (st + 1) * SW],
                    MUL,
                    MUL,
                    accum_out=sums[:, st : st + 1],
                )

            tot = singles.tile([BPC, 1], f32)
            nc.vector.reduce_sum(tot, sums, axis=mybir.AxisListType.X)
            rs = singles.tile([BPC, 1], f32)
            nc.vector.reciprocal(rs, tot)
            outw = singles.tile([BPC, S], f32)
            nc.vector.tensor_scalar_mul(outw, prob, rs)
            nc.sync.dma_start(out=out.ap(), in_=outw)

    nc.finalize()
    return nc


def get_nc():
    if "nc" not in _CACHE:
        _CACHE["nc"] = _build_nc()
    return _CACHE["nc"]


def prep_in_maps(query, keys, mask, W1, W2, v):
    query = np.ascontiguousarray(np.asarray(query, dtype=np.float32))
    keys = np.asarray(keys, dtype=np.float32)
    mask = np.asarray(mask)
    W1 = np.asarray(W1, dtype=np.float32)
    W2 = np.asarray(W2, dtype=np.float32)
    v = np.asarray(v, dtype=np.float32)

    import ml_dtypes

    bf = ml_dtypes.bfloat16
    w1t = np.ascontiguousarray(W1.T).astype(bf)
    w2t = np.ascontiguousarray(W2.T).astype(bf)
    # vsel[p, b, m, j] = v[m*128 + p] if j == b else 0
    vsel = np.zeros((P, BPC, MC, BPC), dtype=np.float32)
    vchunk = v.reshape(MC, P).T  # [p, m]
    for b in range(BPC):
        vsel[:, b, :, b] = vchunk
    vsel = vsel.astype(bf)
    m01f = mask.astype(np.float32)

    in_maps = []
    for c in range(NCORES):
        sl = slice(c * BPC, (c + 1) * BPC)
        in_maps.append(
            {
                "kT": np.ascontiguousarray(keys[sl].transpose(0, 2, 1)).astype(bf),
                "w2t": w2t,
                "w1t": w1t,
                "qT": np.ascontiguousarray(query[sl].T).astype(bf),
                "vsel": vsel,
                "m01": np.ascontiguousarray(m01f[sl]),
            }
        )
    return in_maps


def run(query, keys, mask, W1, W2, v, trace=False):
    """Run on the 8 NeuronCores; returns (output, BassKernelResults)."""
    from concourse.bass_utils import run_bass_kernel_spmd

    nc = get_nc()
    in_maps = prep_in_maps(query, keys, mask, W1, W2, v)
    res = run_bass_kernel_spmd(nc, in_maps, core_ids=list(range(NCORES)), trace=trace)
    outs = [np.asarray(res.results[c]["out"]) for c in range(NCORES)]
    full = np.concatenate(outs, axis=0).astype(np.float32)
    return full, res


def kernel(query, keys, mask, W1, W2, v):
    full, _ = run(query, keys, mask, W1, W2, v, trace=False)
    return full
